# revision 15
# baseline (speedup 1.0000x reference)
"""Bass/Tile TRN2 kernel for nn_BatchGraphAttentionLayer.

Reference computation (per batch b):
    Wh  = h[b] @ W                    # [64, 256]
    s1  = Wh @ a[:256], s2 = Wh @ a[256:]
    e   = leaky_relu(s1[i] + s2[j])   # [64, 64]
    att = softmax over axis i of where(adj[i,j]>0, e, -9e15)
    out = elu(att @ Wh)               # contraction over j

Sharding: data-parallel over batch. 8 cores x 4 batches each.
Each core gets host-pre-transposed hT [16384, 256] (k-major) split into
bf16 hi/lo halves so the projection runs at bf16 PE speed with ~fp32
accuracy (3-term compensated product: hh + hl + lh; the dropped lo*lo
term is O(2^-16) relative).  Same total HBM traffic as fp32.
W is replicated, also split hi/lo.

On-chip layout notes:
  - Projection accumulates Wh in m-major PSUM tiles [128 m, 256 o]
    (m = 4*64 = 256 local rows; two tiles of 128).
  - The 4 batches are processed as 2 "pairs": pair t stacks batches
    (2t, 2t+1) on the 128 partitions.  Attention for a pair is computed
    on a [128, 128] tile whose off-diagonal 64x64 blocks are masked to
    -9e15; after softmax those blocks are exactly 0, so a single
    [128,128] x [128,256] matmul yields both batches' outputs.
  - e^T[j, i] = s2[j] + s1[i] is built with two rank-1 matmuls into
    PSUM (ones (x) s1 and s2 (x) ones).
"""

import os
from contextlib import ExitStack

import ml_dtypes
import numpy as np

import concourse.bass as bass
import concourse.tile as tile
from concourse import bacc, mybir
from concourse.bass_utils import run_bass_kernel_spmd

F32 = mybir.dt.float32
BF16 = mybir.dt.bfloat16

B, N, IN, OUT = 32, 64, 16384, 256
NCORES = 8
BPC = B // NCORES            # batches per core = 4
M = BPC * N                  # local rows = 256
P = 128
NEG = -9e15
ALPHA = 0.2

KSUB = IN // P               # 128 k-subtiles of 128
# slab sizes in k-subtiles; small first slabs so PE starts early
SLABS = [4, 12] + [16] * 7
assert sum(SLABS) == KSUB
SLAB_MAX = max(SLABS)

_NC = None
LAST_EXEC_NS = None
LAST_RESULTS = None


def _build_kernel(ctx: ExitStack, tc: tile.TileContext, out, h_hi, h_lo,
                  w_packA, w_packB, maskmul, maskadd, eye):
    nc = tc.nc

    consts = ctx.enter_context(tc.tile_pool(name="consts", bufs=1))
    hpool = ctx.enter_context(tc.tile_pool(name="hslab", bufs=6))
    wpool = ctx.enter_context(tc.tile_pool(name="wslab", bufs=5))
    whpool = ctx.enter_context(tc.tile_pool(name="wh", bufs=1))
    small = ctx.enter_context(tc.tile_pool(name="small", bufs=2))
    attp = ctx.enter_context(tc.tile_pool(name="att", bufs=2))
    ps_accp = ctx.enter_context(tc.tile_pool(name="psacc", bufs=1, space="PSUM"))
    ps_smallp = ctx.enter_context(tc.tile_pool(name="pssmall", bufs=1, space="PSUM"))
    ps_ep = ctx.enter_context(tc.tile_pool(name="pse", bufs=2, space="PSUM"))
    ps_op = ctx.enter_context(tc.tile_pool(name="pso", bufs=1, space="PSUM"))

    # ---- constants (gpsimd/SWDGE: keep the sync HWDGE queue free for
    # the big streaming loads) ----
    sb_eye = consts.tile([P, P], F32)
    nc.gpsimd.dma_start(sb_eye, eye)
    sb_mm = consts.tile([P, P], F32)
    nc.gpsimd.dma_start(sb_mm, maskmul)
    sb_ma = consts.tile([P, P], F32)
    nc.gpsimd.dma_start(sb_ma, maskadd)
    sb_ones = consts.tile([1, P], F32)
    nc.vector.memset(sb_ones, 1.0)

    # ---- PE warm-up: tiny dummy matmuls so the HAM un-throttles while
    # the first DMA slabs are still in flight ----
    ps_warm = ps_ep.tile([P, P], F32, tag="ps_e", name="ps_warm")
    for i in range(32):
        nc.tensor.matmul(ps_warm, lhsT=sb_ones, rhs=sb_ones,
                         start=(i == 0), stop=(i == 31),
                         skip_group_check=True)

    # ---- phase 1: Wh = h @ W, accumulated in PSUM (m-major) ----
    # 3-term compensated bf16 product: hh + hl + lh.  w_packA carries 4
    # extra columns: [w_hi | wa_hi(2) | wa_lo(2)] where wa = W @ [a1 a2]
    # (host-precomputed), so the attention scores s1/s2 = h @ wa
    # accumulate alongside the projection:
    #   MM_A: stationary h_hi, moving w_packA (260 wide)
    #   MM_B: stationary h_hi, moving w_packB = w_lo (256 wide) -> psB
    #   MM_C: stationary h_lo, moving w_packA[:, 0:258]
    # psA[:, 0:256] = hh+lh, psB = hl, psA[:, 256:258] = h@wa_hi,
    # psA[:, 258:260] = h_hi@wa_lo.
    ps_a = [ps_accp.tile([P, OUT + 4], F32, tag=f"ps_a{t}", name=f"ps_a{t}")
            for t in range(2)]
    ps_b = [ps_accp.tile([P, OUT], F32, tag=f"ps_b{t}", name=f"ps_b{t}")
            for t in range(2)]
    k0 = 0
    for s, nsub in enumerate(SLABS):
        # partition p holds nsub consecutive k-rows -> fully linear DMA.
        # k-subtile c = rows {nsub*p + c}: same k->partition map for h and
        # W, so accumulating over (s, c) contracts every k exactly once.
        ksl = slice(k0 * P, (k0 + nsub) * P)
        hs_hi = hpool.tile([P, SLAB_MAX, M], BF16, tag="hs_hi")
        nc.sync.dma_start(hs_hi[:, :nsub],
                          h_hi[ksl, :].rearrange("(p c) m -> p c m", p=P))
        wsa = wpool.tile([P, SLAB_MAX, OUT + 4], BF16, tag="wsa")
        nc.scalar.dma_start(wsa[:, :nsub],
                            w_packA[ksl, :].rearrange("(p c) o -> p c o", p=P))
        wsb = wpool.tile([P, SLAB_MAX, OUT], BF16, tag="wsb")
        nc.sync.dma_start(wsb[:, :nsub],
                          w_packB[ksl, :].rearrange("(p c) o -> p c o", p=P))
        hs_lo = hpool.tile([P, SLAB_MAX, M], BF16, tag="hs_lo")
        nc.scalar.dma_start(hs_lo[:, :nsub],
                            h_lo[ksl, :].rearrange("(p c) m -> p c m", p=P))
        first = (s == 0)
        last = (s == len(SLABS) - 1)
        for c in range(nsub):
            for t in range(2):
                msl = slice(t * P, (t + 1) * P)
                st = first and c == 0
                sp = last and c == nsub - 1
                nc.tensor.matmul(ps_a[t], lhsT=hs_hi[:, c, msl],
                                 rhs=wsa[:, c, :],
                                 start=st, stop=False,
                                 skip_group_check=True)
                nc.tensor.matmul(ps_b[t], lhsT=hs_hi[:, c, msl],
                                 rhs=wsb[:, c, :],
                                 start=st, stop=sp,
                                 skip_group_check=True)
                nc.tensor.matmul(ps_a[t][:, :OUT + 2], lhsT=hs_lo[:, c, msl],
                                 rhs=wsa[:, c, :OUT + 2],
                                 start=False, stop=sp,
                                 skip_group_check=True)
        k0 += nsub

    # Wh to SBUF (m-major: [m, o]) and score columns
    wh_m = [whpool.tile([P, OUT], F32, tag=f"wh_m{t}", name=f"wh_m{t}")
            for t in range(2)]
    sc = [whpool.tile([P, 2], F32, tag=f"sc{t}", name=f"sc{t}")
          for t in range(2)]
    for t in range(2):
        tmp = attp.tile([P, OUT], F32, tag="wh_tmp")
        nc.vector.tensor_copy(out=tmp, in_=ps_b[t])
        nc.vector.tensor_tensor(wh_m[t], ps_a[t][:, :OUT], tmp,
                                mybir.AluOpType.add)
        tmp2 = attp.tile([P, 2], F32, tag="sc_tmp")
        nc.vector.tensor_copy(out=tmp2, in_=ps_a[t][:, OUT + 2:OUT + 4])
        nc.vector.tensor_tensor(sc[t], ps_a[t][:, OUT:OUT + 2], tmp2,
                                mybir.AluOpType.add)

    # ---- phase 2a: s1 rows via one tiny transpose per pair ----
    s1row = []
    for t in range(2):
        pst = ps_smallp.tile([2, P], F32, tag="ps_small", name=f"ps_tr{t}")
        nc.tensor.transpose(pst, sc[t], sb_eye)
        sr = small.tile([2, P], F32, tag=f"s1row{t}")
        nc.vector.tensor_copy(out=sr, in_=pst)
        s1row.append(sr)

    # ---- phase 2b: per-pair attention ----
    for t in range(2):
        # eT_pre[j~, i~] = s1[i~] (replicated over j~) via rank-1 matmul;
        # s2[j~] enters as a per-partition scalar below.
        ps_e = ps_ep.tile([P, P], F32, tag="ps_e")
        nc.tensor.matmul(ps_e, lhsT=sb_ones, rhs=s1row[t][0:1, :],
                         start=True, stop=True)
        # v = s1[i~]+s2[j~]; leaky = max(v, 0.2v); mask: *maskmul +maskadd
        va = attp.tile([P, P], F32, tag="va")
        nc.vector.tensor_scalar(va, ps_e, sc[t][:, 1:2], None,
                                mybir.AluOpType.add)
        vb = attp.tile([P, P], F32, tag="vb")
        nc.vector.tensor_scalar(vb, ps_e, sc[t][:, 1:2], ALPHA,
                                mybir.AluOpType.add, mybir.AluOpType.mult)
        lk = attp.tile([P, P], F32, tag="lk")
        nc.vector.tensor_tensor(lk, va, vb, mybir.AluOpType.max)
        nc.vector.tensor_tensor(lk, lk, sb_mm, mybir.AluOpType.mult)
        nc.vector.tensor_tensor(lk, lk, sb_ma, mybir.AluOpType.add)
        # softmax along free dim
        nmax = small.tile([P, 1], F32, tag="nmax")
        nc.vector.tensor_reduce(nmax, lk, axis=mybir.AxisListType.X,
                                op=mybir.AluOpType.max, negate=True)
        pexp = attp.tile([P, P], F32, tag="pexp")
        rsum = small.tile([P, 1], F32, tag="rsum")
        nc.scalar.activation(pexp, lk, mybir.ActivationFunctionType.Exp,
                             bias=nmax, scale=1.0, accum_out=rsum)
        rinv = small.tile([P, 1], F32, tag="rinv")
        nc.vector.reciprocal(rinv, rsum)
        att = attp.tile([P, P], F32, tag="att")
        nc.vector.tensor_scalar_mul(att, pexp, rinv)
        # out[i~, o] = sum_j~ att[j~, i~] * Wh[j~, o]  (off-diag blocks are 0)
        ps_o = ps_op.tile([P, OUT], F32, tag="ps_o")
        nc.tensor.matmul(ps_o, lhsT=att, rhs=wh_m[t], start=True, stop=True)
        # elu(x) = max(x,0)-1 + exp(min(x,0))
        m0 = attp.tile([P, OUT], F32, tag="m0")
        nc.vector.tensor_scalar_min(m0, ps_o, 0.0)
        ex = attp.tile([P, OUT], F32, tag="ex")
        nc.scalar.activation(ex, m0, mybir.ActivationFunctionType.Exp)
        rm1 = attp.tile([P, OUT], F32, tag="rm1")
        nc.vector.tensor_scalar(rm1, ps_o, 0.0, -1.0,
                                mybir.AluOpType.max, mybir.AluOpType.add)
        ot = attp.tile([P, OUT], F32, tag="ot")
        nc.vector.tensor_tensor(ot, ex, rm1, mybir.AluOpType.add)
        nc.sync.dma_start(out[t * P:(t + 1) * P, :], ot)


def _get_nc():
    global _NC
    if _NC is not None:
        return _NC
    nc = bacc.Bacc("TRN2", target_bir_lowering=False, debug=False,
                   num_devices=NCORES)
    h_hi = nc.dram_tensor("h_hi", [IN, M], BF16, kind="ExternalInput").ap()
    h_lo = nc.dram_tensor("h_lo", [IN, M], BF16, kind="ExternalInput").ap()
    w_packA = nc.dram_tensor("w_packA", [IN, OUT + 4], BF16,
                             kind="ExternalInput").ap()
    w_packB = nc.dram_tensor("w_packB", [IN, OUT], BF16,
                             kind="ExternalInput").ap()
    maskmul = nc.dram_tensor("maskmul", [P, P], F32, kind="ExternalInput").ap()
    maskadd = nc.dram_tensor("maskadd", [P, P], F32, kind="ExternalInput").ap()
    eye = nc.dram_tensor("eye", [P, P], F32, kind="ExternalInput").ap()
    out = nc.dram_tensor("out", [M, OUT], F32, kind="ExternalOutput").ap()
    with tile.TileContext(nc) as tc:
        with ExitStack() as ctx:
            _build_kernel(ctx, tc, out, h_hi, h_lo, w_packA, w_packB,
                          maskmul, maskadd, eye)
    nc.compile()
    _NC = nc
    return nc


def _masks(adj: np.ndarray):
    adjb = (np.asarray(adj) > 0)                 # [i, j]
    mm = np.zeros((P, P), np.float32)
    mm[:N, :N] = adjb.T.astype(np.float32)       # [j, i]
    mm[N:, N:] = adjb.T.astype(np.float32)
    ma = np.where(mm > 0, np.float32(0.0), np.float32(NEG)).astype(np.float32)
    return mm, ma


def _split_hi_lo(x: np.ndarray):
    hi = x.astype(ml_dtypes.bfloat16)
    lo = (x - hi.astype(np.float32)).astype(ml_dtypes.bfloat16)
    return hi, lo


def kernel(h: np.ndarray, adj: np.ndarray, W: np.ndarray, a: np.ndarray
           ) -> np.ndarray:
    global LAST_EXEC_NS, LAST_RESULTS
    h = np.asarray(h, dtype=np.float32)
    W = np.asarray(W, dtype=np.float32)
    a = np.ascontiguousarray(np.asarray(a, dtype=np.float32)).reshape(2 * OUT, 1)
    assert h.shape == (B, N, IN) and W.shape == (IN, OUT)

    nc = _get_nc()
    mm, ma = _masks(adj)
    eye = np.eye(P, dtype=np.float32)
    w_hi, w_lo = _split_hi_lo(W)
    wa = W.astype(np.float64) @ a.reshape(2, OUT).T.astype(np.float64)
    wa = wa.astype(np.float32)          # [IN, 2] = [W@a1, W@a2]
    wa_hi, wa_lo = _split_hi_lo(wa)
    w_packA = np.ascontiguousarray(np.concatenate(
        [w_hi, wa_hi, wa_lo], axis=1))  # [IN, 260]
    w_packB = np.ascontiguousarray(w_lo)

    in_maps = []
    for c in range(NCORES):
        hT = h[c * BPC:(c + 1) * BPC].reshape(M, IN).T
        h_hi, h_lo = _split_hi_lo(np.ascontiguousarray(hT))
        in_maps.append({"h_hi": h_hi, "h_lo": h_lo, "w_packA": w_packA,
                        "w_packB": w_packB, "maskmul": mm, "maskadd": ma,
                        "eye": eye})

    trace = os.environ.get("GAT_TRACE", "0") == "1"
    res = run_bass_kernel_spmd(nc, in_maps, list(range(NCORES)), trace=trace)
    LAST_EXEC_NS = res.exec_time_ns
    LAST_RESULTS = res

    out = np.empty((B, N, OUT), np.float32)
    for c in range(NCORES):
        out[c * BPC:(c + 1) * BPC] = res.results[c]["out"].reshape(BPC, N, OUT)
    return out
